# revision 2
# baseline (speedup 1.0000x reference)
"""Trainium2 Bass kernel for the depth-2 TT-compressed meta-linear module.

Math (per token t, with x the (D,)-vector of that token, repeated DEPTH=2):
    w0[r]      = sum_d x[d] * core0[0,d,r]
    y1[r,R]    = sum_d x[d] * core1[r,d,R]
    w1[R]      = sum_r w0[r] * y1[r,R]
    y2[r,R]    = sum_d x[d] * core2[r,d,R]
    w2[R]      = sum_r w1[r] * y2[r,R]
    x'[d]      = sum_R w2[R] * core3[R,d,0]
Output = x'' + bias.

Device mapping (8-way data parallel over tokens; 2048 tokens/core):
  - x is pre-transposed AND pre-sharded on the host to a d-major fp16
    layout [128, NCH, T_CORE] (partition = d%128, chunk = d//128), so the
    kernel needs NO on-device transposes and input DMA traffic is halved.
  - Depth 1: one 128-wide matmul pass computes [w0 replicated | y1] (p1);
    a 64-wide pass computes y2 into the BOTTOM half of a shared PSUM bank
    (PB[0:64]); the r-sum SREP matmul writes the TOP half (PB[64:128]).
  - All four z-chain multiplies read BOTH operands directly from one PSUM
    tile (z1 = p1[64:128]*p1[0:64]; z2 = PB[64:128]*PB[0:64]), so the Act
    staging copies of the previous design are gone entirely: per block the
    only Act/DVE work is 4 DVE multiplies + 8 PSUM->SBUF output copies.
  - The depth boundary is linear, so depth 2's input contractions are folded
    on the host: M01 = C3S @ C01 and M2 = C3S @ C2 map z2 (depth-1 state)
    straight to depth-2's [w0|y1] (p1b) and y2 (PB2[0:64]).
  - Final: out_tile = z2d2_chunk^T @ (S2@C3) emits token-major [t, d] tiles;
    PSUM->SBUF copies downconvert to bf16 (bf16 keeps fp32's exponent range;
    outputs are ~1e-15 so fp16 would flush) and halve output DMA traffic.
  - Software pipelining: each block's small matmuls (SREP, M01/M2) are
    interleaved into neighboring blocks' big passes and final matmuls.
    Steady state is PE-bound (~5.97us of matmuls per block) with DMA just
    under it (~5.8us per block in+out), Act/DVE well below.
"""

import numpy as np

import concourse.bacc as bacc
import concourse.tile as tile
import concourse.mybir as mybir
import concourse.bass_utils as bass_utils

import os

N_CORES = 8
B, N, D, R = 4, 4096, 1024, 8
T_TOTAL = B * N              # 16384 tokens
T_CORE = T_TOTAL // N_CORES  # 2048 tokens per core
NCH = D // 128               # 8 d-chunks
TTILES = T_CORE // 128       # 16 token-tiles per core

BS = [int(v) for v in os.environ.get("K_BS", "512,512,512,512").split(",")]
assert sum(BS) == T_CORE
NBLK = len(BS)

F32R = mybir.dt.float32r
F32 = mybir.dt.float32
F16 = mybir.dt.float16
BF16 = mybir.dt.bfloat16

OUTSPLIT = int(os.environ.get("K_OUTSPLIT", "4"))
WARM = int(os.environ.get("K_WARM", "9"))


def _build_program(with_bias=False):
    nc = bacc.Bacc("TRN2", target_bir_lowering=False, debug=False,
                   num_devices=N_CORES)

    zp = 65 if with_bias else 64  # final contraction size (65 = +bias row)

    x_d = nc.dram_tensor("x", [128, NCH, T_CORE], F16, kind="ExternalInput")
    out_d = nc.dram_tensor("out", [128, TTILES, D], BF16,
                           kind="ExternalOutput")
    c01_d = nc.dram_tensor("c01", [128, NCH * 128], F16, kind="ExternalInput")
    c2_d = nc.dram_tensor("c2", [128, NCH * 64], F16, kind="ExternalInput")
    # srep | m01 | m2 packed along the free dim
    sm_d = nc.dram_tensor("sm", [64, 256], F32R, kind="ExternalInput")
    s2c3b_d = nc.dram_tensor("s2c3b", [zp, D], F32R, kind="ExternalInput")

    x_ap = x_d.ap()
    out_ap = out_d.ap()

    with tile.TileContext(nc) as tc:
        with (
            tc.tile_pool(name="consts", bufs=1) as cpool,
            tc.tile_pool(name="xt",
                         bufs=int(os.environ.get("K_XT", "3"))) as pool_xt,
            tc.tile_pool(name="zs",
                         bufs=int(os.environ.get("K_ZS", "8"))) as pool_zs,
            tc.tile_pool(name="outsb",
                         bufs=int(os.environ.get("K_OUT", "4"))) as pool_out,
            tc.tile_pool(name="ps_a", bufs=int(os.environ.get("K_PA", "2")),
                         space="PSUM") as ps_a,
            tc.tile_pool(name="ps_b", bufs=int(os.environ.get("K_PB", "2")),
                         space="PSUM") as ps_b,
            tc.tile_pool(name="ps_f",
                         bufs=int(os.environ.get("K_PF", "4")),
                         space="PSUM") as ps_f,
        ):
            c01_s = cpool.tile([128, NCH * 128], F16, tag="c01")
            c2_s = cpool.tile([128, NCH * 64], F16, tag="c2")
            sm_s = cpool.tile([64, 256], F32R, tag="sm")
            srep_s = sm_s[:, 0:64]
            m01_s = sm_s[:, 64:192]
            m2_s = sm_s[:, 192:256]
            s2c3b_s = cpool.tile([zp, D], F32R, tag="s2c3b")

            # ---- PE p-state warm-up: keep TensorE continuously busy while
            # the first DMAs stream in, so real matmuls start at full clock.
            if WARM:
                warm_s = cpool.tile([128, 512], F16, tag="warm")
                nc.gpsimd.memset(warm_s[:], 0.0)
                for w in range(WARM):
                    pw_ = ps_f.tile([128, 512], F32, tag="pf", name="warmpf")
                    nc.tensor.matmul(pw_[:], warm_s[:, 0:128], warm_s[:],
                                     start=True, stop=True)

            def load_x(st, split):
                tb, t0 = st["tb"], st["t0"]
                xt = pool_xt.tile([128, NCH, tb], F16, tag="xt", name="xt")
                if split:
                    # fine-grained first load, ordered by first use so p1(0)
                    # starts as soon as the first half arrives
                    nc.sync.dma_start(c01_s[:, 0:512], c01_d.ap()[:, 0:512])
                    nc.sync.dma_start(xt[:, 0:4, :], x_ap[:, 0:4, t0:t0 + tb])
                    nc.sync.dma_start(xt[:, 4:8, :],
                                      x_ap[:, 4:8, t0:t0 + tb])
                    nc.sync.dma_start(c01_s[:, 512:1024],
                                      c01_d.ap()[:, 512:1024])
                    nc.sync.dma_start(c2_s[:], c2_d.ap()[:])
                    nc.sync.dma_start(sm_s[:], sm_d.ap()[:])
                elif int(os.environ.get("K_XSPLIT", "1")):
                    nc.sync.dma_start(xt[:, 0:4, :], x_ap[:, 0:4, t0:t0 + tb])
                    nc.sync.dma_start(xt[:, 4:8, :], x_ap[:, 4:8, t0:t0 + tb])
                else:
                    nc.sync.dma_start(xt[:], x_ap[:, :, t0:t0 + tb])
                st["xt"] = xt

            def emit_p1(st, depth):
                """Depth-1: [w0rep|y1] = C01^T x. Depth-2: p1b = M01^T z2."""
                tb = st["tb"]
                p1 = ps_a.tile([128, tb], F32, tag="pa",
                               name=f"p1d{depth}")
                if depth == 1:
                    for j in range(NCH):
                        nc.tensor.matmul(p1[:],
                                         c01_s[:, j * 128:(j + 1) * 128],
                                         st["xt"][:, j, :],
                                         start=(j == 0), stop=(j == NCH - 1))
                else:
                    nc.tensor.matmul(p1[:], m01_s, st["z2_1"][:],
                                     start=True, stop=True)
                st[f"p1_{depth}"] = p1

            def emit_p2(st, depth):
                """y2 (depth 1) / p2b (depth 2) into PB[0:64]; PB[64:128]
                is later filled by the SREP r-sum matmul."""
                tb = st["tb"]
                pb = ps_b.tile([128, tb], F32, tag="pb", name=f"pb{depth}")
                if depth == 1:
                    for j in range(NCH):
                        nc.tensor.matmul(pb[0:64, :],
                                         c2_s[:, j * 64:(j + 1) * 64],
                                         st["xt"][:, j, :],
                                         start=(j == 0), stop=(j == NCH - 1))
                else:
                    nc.tensor.matmul(pb[0:64, :], m2_s, st["z2_1"][:],
                                     start=True, stop=True)
                st[f"pb_{depth}"] = pb

            def chain_z1(st, depth):
                """z1 = w0 * y1, both operands direct from the p1 PSUM tile."""
                tb = st["tb"]
                p1 = st[f"p1_{depth}"]
                z1 = pool_zs.tile([64, tb], F32R, tag=f"z1{depth}",
                                  name=f"z1d{depth}")
                nc.vector.tensor_mul(z1[:], p1[64:128, :], p1[0:64, :])
                st[f"z1_{depth}"] = z1

            def chain_srep(st, depth):
                """r-sum on TensorE into the top half of the shared PB bank."""
                pb = st[f"pb_{depth}"]
                nc.tensor.matmul(pb[64:128, :], srep_s,
                                 st[f"z1_{depth}"][:],
                                 start=True, stop=True)

            def chain_z2(st, depth):
                """z2 = pw * y2, both operands direct from the PB PSUM tile."""
                tb = st["tb"]
                pb = st[f"pb_{depth}"]
                pp = zp if depth == 2 else 64
                z2 = pool_zs.tile([pp, tb], F32R, tag=f"z2{depth}",
                                  name=f"z2d{depth}")
                nc.vector.tensor_mul(z2[0:64, :], pb[64:128, :], pb[0:64, :])
                if depth == 2 and with_bias:
                    nc.vector.memset(z2[64:65, :], 1.0)
                st[f"z2_{depth}"] = z2

            def final_one(st, k, ceng, tail=False):
                """One final matmul [128, 512] + bf16 copy (rotating engine)
                per half-tile; out DMA per OUTSPLIT group of tiles."""
                ntile = st["ntile"]
                i, h = k // 2, k % 2
                if k == 0:
                    st["osb"] = pool_out.tile([128, ntile, D], BF16,
                                              tag="outsb", name="osb")
                z2 = st["z2_2"]
                if tail and int(os.environ.get("K_TWID", "1")):
                    # ps_a/ps_b are idle at the tail: widen the pf ring
                    tp = (ps_f, ps_a, ps_b)[k % 3]
                    tag = {id(ps_f): "pf", id(ps_a): "pa",
                           id(ps_b): "pb"}[id(tp)]
                    pf = tp.tile([128, 512], F32, tag=tag, name="pf")
                else:
                    pf = ps_f.tile([128, 512], F32, tag="pf", name="pf")
                nc.tensor.matmul(pf[:], z2[:, i * 128:(i + 1) * 128],
                                 s2c3b_s[:, h * 512:(h + 1) * 512],
                                 start=True, stop=True)
                dst = st["osb"][:, i, h * 512:(h + 1) * 512]
                if ceng == 0:
                    nc.scalar.copy(dst, pf[:])
                else:
                    nc.vector.tensor_copy(dst, pf[:])
                # out DMA per OUTSPLIT group of tiles
                g0 = st["g0"]
                if OUTSPLIT > ntile:
                    nc.sync.dma_start(
                        out_ap[:, g0 + i:g0 + i + 1, h * 512:(h + 1) * 512],
                        st["osb"][:, i:i + 1, h * 512:(h + 1) * 512])
                    return
                osp = min(OUTSPLIT, ntile)
                if h == 1 and (i + 1) % (ntile // osp) == 0:
                    i0g = i + 1 - ntile // osp
                    deng = nc.sync
                    if tail and i % 2 == 1 and int(os.environ.get("K_TDQ",
                                                                  "0")):
                        deng = nc.scalar
                    deng.dma_start(out_ap[:, g0 + i0g:g0 + i + 1, :],
                                   st["osb"][:, i0g:i + 1, :])

            # ---- software-pipelined emission --------------------------------
            state = []
            t0 = 0
            for b, tb in enumerate(BS):
                state.append({"tb": tb, "t0": t0, "g0": t0 // 128,
                              "ntile": tb // 128, "nf": tb // 64})
                t0 += tb
            load_x(state[0], split=True)
            if NBLK > 1:
                load_x(state[1], split=False)
            nc.sync.dma_start(s2c3b_s[:], s2c3b_d.ap()[:])

            # copy-engine rotation (0=Act, 2=DVE)
            CROT = [int(c) for c in os.environ.get("K_CROT", "20020000")]

            for b in range(NBLK + 1):
                st = state[b] if b < NBLK else None
                pv = state[b - 1] if b >= 1 else None

                if st is not None:
                    if b + 2 < NBLK:
                        load_x(state[b + 2], split=False)
                    emit_p1(st, 1)                   # PE: 8 mm
                if pv is not None:
                    chain_srep(pv, 2)                # PE (needs z1b(pv))
                    chain_z2(pv, 2)                  # DVE -> z2b(pv)
                if st is not None:
                    emit_p2(st, 1)                   # PE: 8 mm -> PB[0:64]
                    chain_z1(st, 1)                  # DVE (needs p1)

                # interleave positions for block-b stages inside finals(pv)
                NFv = st["nf"] if st is not None else 0
                KSR = int(os.environ.get("K_KSR", "1"))
                KMD = int(os.environ.get("K_KMD", "5"))
                if b == NBLK - 1:
                    KSR = int(os.environ.get("K_KSRL", "1"))
                    KMD = int(os.environ.get("K_KMDL", "5"))
                srp = {}
                if st is not None:
                    srp[min(KSR, NFv - 1)] = "srep"
                    srp[min(KMD, NFv - 1) if KMD < NFv or NFv == 0
                        else NFv - 1] = "mid"
                done = set()

                def blk_stage(k):
                    if st is None or k not in srp or k in done:
                        return
                    done.add(k)
                    if srp[k] == "srep":
                        chain_srep(st, 1)            # PE: srep1
                        chain_z2(st, 1)              # DVE -> z2(st)
                    else:
                        emit_p1(st, 2)               # PE: m01
                        emit_p2(st, 2)               # PE: m2 -> PB2[0:64]
                        chain_z1(st, 2)              # DVE -> z1b(st)

                if pv is not None:
                    tail = b == NBLK
                    for k in range(pv["nf"]):
                        blk_stage(k)
                        ce = (2, 0)[(k + int(os.environ.get("K_TCE", "0")))
                                    % 2] if tail else CROT[k % len(CROT)]
                        final_one(pv, k, ce, tail=tail)
                for k in sorted(srp):
                    if k not in done:
                        blk_stage(k)

    nc.compile()
    return nc


def _constants(core0, core1, core2, core3, bias):
    core0 = np.asarray(core0, np.float64)
    core1 = np.asarray(core1, np.float64)
    core2 = np.asarray(core2, np.float64)
    core3 = np.asarray(core3, np.float64)
    bias = np.asarray(bias, np.float64)

    # k index = r*8 + R  (prev rank r, next rank R)
    C01 = np.zeros((D, 128))
    C01[:, :64] = np.repeat(core0[0], R, axis=1)           # w0 replicated in R
    C01[:, 64:] = core1.transpose(1, 0, 2).reshape(D, 64)  # y1
    C2 = core2.transpose(1, 0, 2).reshape(D, 64)
    SREP = np.kron(np.ones((R, 1)),
                   np.kron(np.eye(R), np.ones((1, R))))    # (64,64)
    S2 = np.tile(np.eye(R), (R, 1))                        # (64,8)
    C3S = np.tile(core3[:, :, 0], (R, 1))                  # (64,D)
    # host-folded depth boundary
    M01 = C3S @ C01
    M2 = C3S @ C2
    S2C3B = S2 @ core3[:, :, 0]                            # (64,D)

    def chunk_major(a, po):
        # (D, po) -> (128, NCH*po) with d-chunk along the free dim
        return np.ascontiguousarray(
            a.reshape(NCH, 128, po).transpose(1, 0, 2).reshape(128, NCH * po))

    with_bias = bool(np.any(bias))
    # matmul computes lhsT.T @ rhs with lhsT=[K, M]; M01 [64, 128] already has
    # K=z2-components on rows, so p1b = M01.T @ z2_dmaj as required.
    sm = np.concatenate([SREP, M01, M2], axis=1)

    if with_bias:
        s2c3b = np.concatenate([S2C3B, bias[None, :]], axis=0)
    else:
        s2c3b = S2C3B
    s2c3b = np.ascontiguousarray(s2c3b).astype(np.float32)

    consts = {
        "c01": chunk_major(C01, 128).astype(np.float16),
        "c2": chunk_major(C2, 64).astype(np.float16),
        "sm": np.ascontiguousarray(sm).astype(np.float32),
        "s2c3b": s2c3b,
    }
    return consts, with_bias


_NC_CACHE = {}


def _get_program(with_bias=False):
    if with_bias not in _NC_CACHE:
        _NC_CACHE[with_bias] = _build_program(with_bias)
    return _NC_CACHE[with_bias]


def run(x, core0, core1, core2, core3, bias, trace=False, **spmd_kwargs):
    consts, with_bias = _constants(core0, core1, core2, core3, bias)
    nc = _get_program(with_bias)
    xf = np.asarray(x, np.float32).reshape(T_TOTAL, D).astype(np.float16)
    in_maps = []
    for c in range(N_CORES):
        m = dict(consts)
        xc = xf[c * T_CORE:(c + 1) * T_CORE]               # [T_CORE, D]
        # d-major [128, NCH, T_CORE]: partition p=d%128, chunk j=d//128
        xd = np.ascontiguousarray(
            xc.T.reshape(NCH, 128, T_CORE).transpose(1, 0, 2))
        m["x"] = xd
        in_maps.append(m)
    res = bass_utils.run_bass_kernel_spmd(
        nc, in_maps, core_ids=list(range(N_CORES)), trace=trace, **spmd_kwargs)
    outs = []
    for c in range(N_CORES):
        oc = np.asarray(res.results[c]["out"])    # [128, TTILES, D] bf16
        # token t = g*128 + p
        outs.append(oc.transpose(1, 0, 2).reshape(T_CORE, D))
    out = np.concatenate(outs, axis=0).astype(np.float32)
    return out.reshape(B, N, D), res


def kernel(x, core0, core1, core2, core3, bias):
    out, _ = run(x, core0, core1, core2, core3, bias)
    return out
